# revision 9
# baseline (speedup 1.0000x reference)
"""Betti-matching-loss preprocessing kernel for 8 TRN2 NeuronCores.

Reference computation (per full input of shape (B=4, C=1, D=128, H=256, W=256)):
    pred_super   = 1 - maxpool3d_2x(sigmoid(input))   -> sigmoid is monotone, so
                 = sigmoid(-maxpool3d_2x(input))
    target_super = 1 - (maxpool3d_2x(target) > 0.5)   = (maxpool3d_2x(target) <= 0.5)
    out = stack([pred_super, target_super])           # (2, B, C, 64, 128, 128)

Sharding: pure data parallel. 8 shards = 4 batch samples x 2 D-halves of 64
planes each (the D split at an even index never crosses a pool window).

Per-core kernel (v3):
  * Loads are 1 MB (4 planes), laid out so partition p holds rows (2p, 2p+1)
    of each plane (2 KB contiguous descriptors).  Input-tensor loads issue
    from the SP (sync) HW DMA queue and target-tensor loads from the ACT
    (scalar) HW DMA queue: a single queue's template generation (~5.3 ns per
    2 KB descriptor) plus head-of-line blocking on pool-recycle waits
    throttled the v1 kernel to ~250-290 GB/s in the back half of the window.
  * A deep load pool (16 x 8 KB/partition) keeps triggers far ahead of the
    DVE pipeline so recycle semaphores never gate the queues.  First and
    last chunks are halved (2 planes) to shorten ramp-up and drain-down.
  * 3-level pairwise tensor_max tree pools D, H, W on DVE; sigmoid (ACT)
    writes bf16 and the <=0.5 compare (DVE) writes uint8 straight into two
    SBUF staging tiles holding the core's whole output (h-major).  bf16 is
    ~2^-9 relative error on the sigmoid plane (gate is 2e-2) and uint8 is
    exact for the 0/1 target plane.
  * Staging is stored to HBM in z-slices as soon as the slice's last chunk
    is pooled, so the DVE drain-down overlaps store traffic and only a
    0.4 MB slice remains after the final compute.  The host transposes
    h-major back to (z, h, w) and upcasts to f32.
"""

import numpy as np

import bass_rust
import concourse.bass as bass
import concourse.mybir as mybir
import concourse.tile as tile
from concourse.bass_utils import run_bass_kernel_spmd
from concourse.vector_clock import ScopedClock

f32 = mybir.dt.float32
bf16 = mybir.dt.bfloat16
u8 = mybir.dt.uint8


def _patched_drain_and_barrier(self, tick_clock, wait_clock):
    """Replacement for TileContext._drain_and_barrier.

    The stock version hangs every outstanding semaphore wait on one Drain
    instruction; the walrus in this environment rejects >1 sync-wait per
    non-EventSemaphore instruction ("Too many sync wait commands").  Emit
    one sequencer NOP per semaphore wait instead, then drain + barrier.
    """
    ((_, vclock),) = ScopedClock({None: tick_clock.global_clock}).items()
    ticks = list(vclock)
    for proc_idx, sem in self.sems.allocated().items():
        t = ticks[proc_idx]
        if t > 0:
            self.nc.sync.nop()._wait_ge(sem, bass_rust.tick_to_sem(t, proc_idx))
    self.nc.sync.drain()
    self.nc.all_engine_barrier(sem_only=True)
    popped = self.nc._tile_sem_poison_stack.pop()
    assert popped is self._sem_poison
    self.nc.clear_and_free_semaphores(list(self.sems.allocated().values()))


tile.TileContext._drain_and_barrier = _patched_drain_and_barrier


def _split_excess_waits(nc: bass.Bass) -> None:
    """Walrus in this env caps sync-waits at 1 per instruction (2 for
    EventSemaphore).  Move excess waits onto same-engine NoOps inserted
    immediately before the offending instruction."""
    for f in nc.m.functions:
        for bb in f.blocks:
            insts = bb.instructions
            out = []
            changed = False
            for inst in insts:
                si = inst.sync_info
                cap = 2 if type(inst).__name__ == "InstEventSemaphore" else 1
                if si is not None and len(si.on_wait) > cap:
                    w = list(si.on_wait)
                    for k, extra in enumerate(w[cap:]):
                        nop = mybir.InstNoOp(
                            name=f"{inst.name}-xw{k}",
                            engine=inst.engine,
                            sync_info=mybir.SyncInfo(
                                on_wait=[extra], on_update=[]
                            ),
                            bass_nofuse=True,
                        )
                        nc.register_instruction(nop, overwrite=True)
                        out.append(nop)
                    inst.sync_info = mybir.SyncInfo(
                        on_wait=w[:cap], on_update=si.on_update
                    )
                    changed = True
                out.append(inst)
            if changed:
                bb.instructions = out

B, C, D, H, W = 4, 1, 128, 256, 256
NCORES = 8
D_SH = D // 2      # 64 input planes per core
DZ = D_SH // 2     # 32 output planes per core
HO, WO = H // 2, W // 2
PPT = 4            # input planes per full load tile (1 MB DMAs)
LOAD_BUFS = 20
# z boundaries of the staged output slices stored to HBM
Z_FLUSH = (8, 16, 24, 30, 32)


def build_nc(d_sh: int = D_SH, ppt: int = PPT) -> bass.Bass:
    dz = d_sh // 2
    nc = bass.Bass()
    inp = nc.declare_dram_parameter("input", [d_sh, H, W], f32, isOutput=False)
    tgt = nc.declare_dram_parameter("target", [d_sh, H, W], f32, isOutput=False)
    # h-major outputs: [h, z, w]; host transposes back to (z, h, w)
    outp = nc.declare_dram_parameter("outp", [HO, dz, WO], bf16, isOutput=True)
    outt = nc.declare_dram_parameter("outt", [HO, dz, WO], u8, isOutput=True)

    # chunk schedule: halved first chunks shorten the DMA ramp-up; the tail
    # stays full-size (small tail chunks inflate DVE fixed costs exactly
    # where DVE latency sets the drain-down)
    chunks = [(0, 2), (2, 2)]
    chunks += [(4 + q * ppt, ppt) for q in range((d_sh - 4) // ppt)]
    assert sum(c[1] for c in chunks) == d_sh

    with tile.TileContext(nc) as tc:
        with (
            tc.tile_pool(name="load", bufs=LOAD_BUFS) as load_pool,
            tc.tile_pool(name="lvl1", bufs=3) as pool1,
            tc.tile_pool(name="lvl2", bufs=3) as pool2,
            tc.tile_pool(name="lvl3", bufs=3) as pool3,
            tc.tile_pool(name="stage", bufs=1) as stage_pool,
        ):
            stg_pred = stage_pool.tile([128, dz * WO], bf16, tag="stgp")
            stg_tgt = stage_pool.tile([128, dz * WO], u8, tag="stgt")
            stg_pred_v = stg_pred.rearrange("p (z w) -> p z w", w=WO)
            stg_tgt_v = stg_tgt.rearrange("p (z w) -> p z w", w=WO)

            for d0, cs in chunks:
                zt_q = cs // 2
                z0 = d0 // 2
                for which, src in ((0, inp), (1, tgt)):
                    # ---- load cs planes; partition p <- rows (2p, 2p+1) ----
                    # input loads on the SP queue, target loads on the ACT
                    # queue: two HW DGEs generate descriptors in parallel.
                    t = load_pool.tile([128, ppt * 512], f32, tag="load")
                    sv = src[d0:d0 + cs].rearrange(
                        "d (h2 hp) w -> h2 d hp w", hp=2
                    )
                    dv = t.rearrange("p (d hp w) -> p d hp w", d=ppt, hp=2)[
                        :, :cs
                    ]
                    eng = nc.sync if which == 0 else nc.scalar
                    eng.dma_start(dv, sv)

                    # ---- level 1: pool D (pairs of planes) ----
                    # (this walrus only codegens TensorTensor on DVE)
                    u = pool1.tile([128, (ppt // 2) * 512], f32, tag="u")
                    tv = t.rearrange("p (z two blk) -> p z two blk", two=2, blk=512)
                    nc.vector.tensor_max(
                        u.rearrange("p (z blk) -> p z blk", blk=512)[:, :zt_q],
                        tv[:, :zt_q, 0, :],
                        tv[:, :zt_q, 1, :],
                    )

                    # ---- level 2: pool H (row 2p vs 2p+1, free-dim halves) ----
                    v = pool2.tile([128, (ppt // 2) * 256], f32, tag="v")
                    uv = u.rearrange("p (z hp w) -> p z hp w", hp=2, w=256)
                    nc.vector.tensor_max(
                        v.rearrange("p (z w) -> p z w", w=256)[:, :zt_q],
                        uv[:, :zt_q, 0, :],
                        uv[:, :zt_q, 1, :],
                    )

                    # ---- level 3: pool W (even/odd columns) ----
                    o = pool3.tile([128, (ppt // 2) * 128], f32, tag="o")
                    vv = v.rearrange("p (z w two) -> p z w two", w=WO, two=2)
                    nc.vector.tensor_max(
                        o.rearrange("p (z w) -> p z w", w=128)[:, :zt_q],
                        vv[:, :zt_q, :, 0],
                        vv[:, :zt_q, :, 1],
                    )

                    # ---- pointwise, straight into the staging tiles ----
                    if which == 0:
                        nc.scalar.activation(
                            stg_pred[:, z0 * WO:(z0 + zt_q) * WO],
                            o[:, :zt_q * 128],
                            mybir.ActivationFunctionType.Sigmoid,
                            bias=0.0, scale=-1.0,
                        )
                    else:
                        nc.vector.tensor_scalar(
                            stg_tgt[:, z0 * WO:(z0 + zt_q) * WO],
                            o[:, :zt_q * 128],
                            0.5, None, mybir.AluOpType.is_le,
                        )

            # staged-output stores, emitted after every load trigger in
            # program order so they never head-of-line-block the load
            # queues; finished z-slices have their semaphores already met,
            # so the scheduler can hoist them into the drain-down window
            a = 0
            for b in Z_FLUSH:
                nc.sync.dma_start(outp[:, a:b], stg_pred_v[:, a:b])
                nc.scalar.dma_start(outt[:, a:b], stg_tgt_v[:, a:b])
                a = b
    _split_excess_waits(nc)
    return nc


_NC_CACHE: dict = {}


def kernel(input: np.ndarray, target: np.ndarray) -> np.ndarray:
    input = np.asarray(input, dtype=np.float32)
    target = np.asarray(target, dtype=np.float32)
    assert input.shape == (B, C, D, H, W), input.shape

    if "nc" not in _NC_CACHE:
        _NC_CACHE["nc"] = build_nc()
    nc = _NC_CACHE["nc"]

    in_maps = []
    for i in range(NCORES):
        b, half = divmod(i, 2)
        sl = slice(half * D_SH, (half + 1) * D_SH)
        in_maps.append({
            "input": np.ascontiguousarray(input[b, 0, sl]),
            "target": np.ascontiguousarray(target[b, 0, sl]),
        })

    res = run_bass_kernel_spmd(nc, in_maps, core_ids=list(range(NCORES))).results

    full = np.empty((2, B, C, D // 2, HO, WO), dtype=np.float32)
    for i in range(NCORES):
        b, half = divmod(i, 2)
        zsl = slice(half * DZ, (half + 1) * DZ)
        # outputs are (h, z, w) -> (z, h, w), upcast to f32
        full[0, b, 0, zsl] = (
            np.asarray(res[i]["outp"]).astype(np.float32).transpose(1, 0, 2)
        )
        full[1, b, 0, zsl] = (
            np.asarray(res[i]["outt"]).astype(np.float32).transpose(1, 0, 2)
        )
    return full


# revision 12
# speedup vs baseline: 1.0303x; 1.0303x over previous
"""Betti-matching-loss preprocessing kernel for 8 TRN2 NeuronCores.

Reference computation (per full input of shape (B=4, C=1, D=128, H=256, W=256)):
    pred_super   = 1 - maxpool3d_2x(sigmoid(input))   -> sigmoid is monotone, so
                 = sigmoid(-maxpool3d_2x(input))
    target_super = 1 - (maxpool3d_2x(target) > 0.5)   = (maxpool3d_2x(target) <= 0.5)
    out = stack([pred_super, target_super])           # (2, B, C, 64, 128, 128)

Sharding: pure data parallel. 8 shards = 4 batch samples x 2 D-halves of 64
planes each (the D split at an even index never crosses a pool window).

Per-core kernel (v3):
  * Loads are 1 MB (4 planes), laid out so partition p holds rows (2p, 2p+1)
    of each plane (2 KB contiguous descriptors).  Input-tensor loads issue
    from the SP (sync) HW DMA queue and target-tensor loads from the ACT
    (scalar) HW DMA queue: a single queue's template generation (~5.3 ns per
    2 KB descriptor) plus head-of-line blocking on pool-recycle waits
    throttled the v1 kernel to ~250-290 GB/s in the back half of the window.
  * A deep load pool (16 x 8 KB/partition) keeps triggers far ahead of the
    DVE pipeline so recycle semaphores never gate the queues.  First and
    last chunks are halved (2 planes) to shorten ramp-up and drain-down.
  * 3-level pairwise tensor_max tree pools D, H, W on DVE; sigmoid (ACT)
    writes bf16 and the <=0.5 compare (DVE) writes uint8 straight into two
    SBUF staging tiles holding the core's whole output (h-major).  bf16 is
    ~2^-9 relative error on the sigmoid plane (gate is 2e-2) and uint8 is
    exact for the 0/1 target plane.
  * Staging is stored to HBM in z-slices as soon as the slice's last chunk
    is pooled, so the DVE drain-down overlaps store traffic and only a
    0.4 MB slice remains after the final compute.  The host transposes
    h-major back to (z, h, w) and upcasts to f32.
"""

import numpy as np

import bass_rust
import concourse.bass as bass
import concourse.mybir as mybir
import concourse.tile as tile
from concourse.bass_utils import run_bass_kernel_spmd
from concourse.vector_clock import ScopedClock

f32 = mybir.dt.float32
bf16 = mybir.dt.bfloat16
u8 = mybir.dt.uint8


def _patched_drain_and_barrier(self, tick_clock, wait_clock):
    """Replacement for TileContext._drain_and_barrier.

    The stock version hangs every outstanding semaphore wait on one Drain
    instruction; the walrus in this environment rejects >1 sync-wait per
    non-EventSemaphore instruction ("Too many sync wait commands").  Emit
    one sequencer NOP per semaphore wait instead, then drain + barrier.
    """
    ((_, vclock),) = ScopedClock({None: tick_clock.global_clock}).items()
    ticks = list(vclock)
    for proc_idx, sem in self.sems.allocated().items():
        t = ticks[proc_idx]
        if t > 0:
            self.nc.sync.nop()._wait_ge(sem, bass_rust.tick_to_sem(t, proc_idx))
    self.nc.sync.drain()
    self.nc.all_engine_barrier(sem_only=True)
    popped = self.nc._tile_sem_poison_stack.pop()
    assert popped is self._sem_poison
    self.nc.clear_and_free_semaphores(list(self.sems.allocated().values()))


tile.TileContext._drain_and_barrier = _patched_drain_and_barrier


def _split_excess_waits(nc: bass.Bass) -> None:
    """Walrus in this env caps sync-waits at 1 per instruction (2 for
    EventSemaphore).  Move excess waits onto same-engine NoOps inserted
    immediately before the offending instruction."""
    for f in nc.m.functions:
        for bb in f.blocks:
            insts = bb.instructions
            out = []
            changed = False
            for inst in insts:
                si = inst.sync_info
                cap = 2 if type(inst).__name__ == "InstEventSemaphore" else 1
                if si is not None and len(si.on_wait) > cap:
                    w = list(si.on_wait)
                    for k, extra in enumerate(w[cap:]):
                        nop = mybir.InstNoOp(
                            name=f"{inst.name}-xw{k}",
                            engine=inst.engine,
                            sync_info=mybir.SyncInfo(
                                on_wait=[extra], on_update=[]
                            ),
                            bass_nofuse=True,
                        )
                        nc.register_instruction(nop, overwrite=True)
                        out.append(nop)
                    inst.sync_info = mybir.SyncInfo(
                        on_wait=w[:cap], on_update=si.on_update
                    )
                    changed = True
                out.append(inst)
            if changed:
                bb.instructions = out

B, C, D, H, W = 4, 1, 128, 256, 256
NCORES = 8
D_SH = D // 2      # 64 input planes per core
DZ = D_SH // 2     # 32 output planes per core
HO, WO = H // 2, W // 2
PPT = 4            # input planes per full load tile (1 MB DMAs)
LOAD_BUFS = 20
# z boundaries of the staged output slices stored to HBM
Z_FLUSH = (8, 16, 24, 30, 32)


def build_nc(d_sh: int = D_SH, ppt: int = PPT) -> bass.Bass:
    dz = d_sh // 2
    nc = bass.Bass()
    inp = nc.declare_dram_parameter("input", [d_sh, H, W], f32, isOutput=False)
    tgt = nc.declare_dram_parameter("target", [d_sh, H, W], f32, isOutput=False)
    # h-major outputs: [h, z, w]; host transposes back to (z, h, w)
    outp = nc.declare_dram_parameter("outp", [HO, dz, WO], bf16, isOutput=True)
    outt = nc.declare_dram_parameter("outt", [HO, dz, WO], u8, isOutput=True)

    # chunk schedule: halved first chunks shorten the DMA ramp-up; halved
    # last chunks minimize the compute that depends on the final loads
    # (which is exactly the drain-down latency)
    chunks = [(0, 2), (2, 2)]
    chunks += [(4 + q * ppt, ppt) for q in range((d_sh - 8) // ppt)]
    chunks += [(d_sh - 4, 2), (d_sh - 2, 2)]
    assert sum(c[1] for c in chunks) == d_sh

    with tile.TileContext(nc) as tc:
        with (
            tc.tile_pool(name="load", bufs=LOAD_BUFS) as load_pool,
            tc.tile_pool(name="lvl1", bufs=3) as pool1,
            tc.tile_pool(name="lvl2", bufs=3) as pool2,
            tc.tile_pool(name="lvl3", bufs=3) as pool3,
            tc.tile_pool(name="stage", bufs=1) as stage_pool,
        ):
            stg_pred = stage_pool.tile([128, dz * WO], bf16, tag="stgp")
            stg_tgt = stage_pool.tile([128, dz * WO], u8, tag="stgt")
            stg_pred_v = stg_pred.rearrange("p (z w) -> p z w", w=WO)
            stg_tgt_v = stg_tgt.rearrange("p (z w) -> p z w", w=WO)
            # bias for the binarizing step activation: sigmoid(-65536*x +
            # 32768) rounded to uint8 {0,1} is exactly (x <= 0.5) for
            # x != 0.5 (uint8 round-to-nearest thresholds sigmoid at 0.5,
            # i.e. the argument at 0) -- this moves the compare from the
            # DVE (on the critical pooling path) to the idle ACT engine
            step_bias = stage_pool.tile([128, 1], f32, tag="sbias")
            nc.vector.memset(step_bias, 32768.0)

            for d0, cs in chunks:
                zt_q = cs // 2
                z0 = d0 // 2
                for which, src in ((0, inp), (1, tgt)):
                    # ---- load cs planes; partition p <- rows (2p, 2p+1) ----
                    # input loads on the SP queue, target loads on the ACT
                    # queue: two HW DGEs generate descriptors in parallel.
                    t = load_pool.tile([128, ppt * 512], f32, tag="load")
                    sv = src[d0:d0 + cs].rearrange(
                        "d (h2 hp) w -> h2 d hp w", hp=2
                    )
                    dv = t.rearrange("p (d hp w) -> p d hp w", d=ppt, hp=2)[
                        :, :cs
                    ]
                    eng = nc.sync if which == 0 else nc.scalar
                    eng.dma_start(dv, sv)

                    # ---- level 1: pool D (pairs of planes) ----
                    # (this walrus only codegens TensorTensor on DVE)
                    u = pool1.tile([128, (ppt // 2) * 512], f32, tag="u")
                    tv = t.rearrange("p (z two blk) -> p z two blk", two=2, blk=512)
                    nc.vector.tensor_max(
                        u.rearrange("p (z blk) -> p z blk", blk=512)[:, :zt_q],
                        tv[:, :zt_q, 0, :],
                        tv[:, :zt_q, 1, :],
                    )

                    # ---- level 2: pool H (row 2p vs 2p+1, free-dim halves) ----
                    v = pool2.tile([128, (ppt // 2) * 256], f32, tag="v")
                    uv = u.rearrange("p (z hp w) -> p z hp w", hp=2, w=256)
                    nc.vector.tensor_max(
                        v.rearrange("p (z w) -> p z w", w=256)[:, :zt_q],
                        uv[:, :zt_q, 0, :],
                        uv[:, :zt_q, 1, :],
                    )

                    # ---- level 3: pool W (even/odd columns) ----
                    o = pool3.tile([128, (ppt // 2) * 128], f32, tag="o")
                    vv = v.rearrange("p (z w two) -> p z w two", w=WO, two=2)
                    nc.vector.tensor_max(
                        o.rearrange("p (z w) -> p z w", w=128)[:, :zt_q],
                        vv[:, :zt_q, :, 0],
                        vv[:, :zt_q, :, 1],
                    )

                    # ---- pointwise, straight into the staging tiles ----
                    if which == 0:
                        nc.scalar.activation(
                            stg_pred[:, z0 * WO:(z0 + zt_q) * WO],
                            o[:, :zt_q * 128],
                            mybir.ActivationFunctionType.Sigmoid,
                            bias=0.0, scale=-1.0,
                        )
                    else:
                        nc.scalar.activation(
                            stg_tgt[:, z0 * WO:(z0 + zt_q) * WO],
                            o[:, :zt_q * 128],
                            mybir.ActivationFunctionType.Sigmoid,
                            bias=step_bias, scale=-65536.0,
                        )

            # staged-output stores, emitted after every load trigger in
            # program order so they never head-of-line-block the load
            # queues; finished z-slices have their semaphores already met,
            # so the scheduler can hoist them into the drain-down window
            a = 0
            for b in Z_FLUSH:
                nc.sync.dma_start(outp[:, a:b], stg_pred_v[:, a:b])
                nc.scalar.dma_start(outt[:, a:b], stg_tgt_v[:, a:b])
                a = b
    _split_excess_waits(nc)
    return nc


_NC_CACHE: dict = {}


def kernel(input: np.ndarray, target: np.ndarray) -> np.ndarray:
    input = np.asarray(input, dtype=np.float32)
    target = np.asarray(target, dtype=np.float32)
    assert input.shape == (B, C, D, H, W), input.shape

    if "nc" not in _NC_CACHE:
        _NC_CACHE["nc"] = build_nc()
    nc = _NC_CACHE["nc"]

    in_maps = []
    for i in range(NCORES):
        b, half = divmod(i, 2)
        sl = slice(half * D_SH, (half + 1) * D_SH)
        in_maps.append({
            "input": np.ascontiguousarray(input[b, 0, sl]),
            "target": np.ascontiguousarray(target[b, 0, sl]),
        })

    res = run_bass_kernel_spmd(nc, in_maps, core_ids=list(range(NCORES))).results

    full = np.empty((2, B, C, D // 2, HO, WO), dtype=np.float32)
    for i in range(NCORES):
        b, half = divmod(i, 2)
        zsl = slice(half * DZ, (half + 1) * DZ)
        # outputs are (h, z, w) -> (z, h, w), upcast to f32
        full[0, b, 0, zsl] = (
            np.asarray(res[i]["outp"]).astype(np.float32).transpose(1, 0, 2)
        )
        full[1, b, 0, zsl] = (
            np.asarray(res[i]["outt"]).astype(np.float32).transpose(1, 0, 2)
        )
    return full
